# revision 1
# baseline (speedup 1.0000x reference)
"""Trainium2 Bass kernel for nn_ASD_RNN (encoder + fused-gate LSTM + prototype-distance head).

Contract: kernel(**inputs) takes FULL unsharded inputs (as in reference.setup_inputs())
and returns the FULL [64, 1] float32 output. Internally shards batch across 8 cores.

Layout notes (per core, BC = 8 batches, R = BC*S = 512 rows, s-major: r = s*8 + b):
  - v rows are PE-transposed on device into vT [f_part, ko, r] for all F-contraction GEMMs.
  - encoder produces fuseT [h_part, m, r]; xg GEMM produces x_gates rows in DRAM scratch.
  - LSTM: hT kept transposed [h_part, k, b]; per-step gates = hT.T@Wh (+ xg via
    identity-matmul fold with an extra ones-row adding bx+bh).
  - prototype distances via ||v||^2 - 2 v.p + ||p||^2 with matmul folds; the W_gate
    column rides along as column 20 of the "pw" matrix.
"""

import numpy as np

import concourse.bass as bass
import concourse.mybir as mybir
import concourse.tile as tile
from concourse import bacc
from concourse.bass_utils import run_bass_kernel_spmd

AF = mybir.ActivationFunctionType
ALU = mybir.AluOpType
DT = mybir.dt
AX = mybir.AxisListType

B, S, F, H, P2 = 64, 64, 2048, 512, 20
G = 4 * H
NCORES = 8
BC = B // NCORES          # 8 batches per core
R = BC * S                # 512 rows per core
KF = F // 128             # 16 k-tiles over F
KH = H // 128             # 4 k-tiles over H
MR = R // 128             # 4 row tiles
F32 = DT.float32

# matmul streaming dtype: float32r = fp32 bits, 4x faster streaming at N>=256
MM_DT = DT.float32r


def _mm(ap):
    return ap


def build_nc():
    nc = bacc.Bacc("TRN2", target_bir_lowering=False, debug=False,
                   num_devices=NCORES)

    def din(name, shape, dt=F32):
        return nc.dram_tensor(name, shape, dt, kind="ExternalInput").ap()

    v_d = din("v", [R, F])
    catrep_d = din("catrep", [1, R])
    iota3_d = din("iota3", [3, 1])
    sel8_d = din("sel8", [128, BC], MM_DT)
    e9_d = din("e9", [9, BC], MM_DT)
    eye21_d = din("eye21", [21, 21])
    eye128_d = din("eye128", [128, 128])
    pw_d = din("pw", [21, F])
    wenc_d = din("wenc", [F, H], MM_DT)
    benc_d = din("benc", [128, KH])
    wx_d = din("wx", [H, G], MM_DT)
    bx_d = din("bx", [1, G])
    bh_d = din("bh", [1, G])
    wh_d = din("wh", [H, G], MM_DT)
    catemb_d = din("catemb", [3, H], MM_DT)
    wdec_d = din("wdec", [KH, 128], MM_DT)
    wddr_d = din("wddrep", [128, MR, P2])
    b3_d = din("b3", [1, 3])
    out_d = nc.dram_tensor("out", [BC, 1], F32, kind="ExternalOutput").ap()
    # internal scratch
    xg3_d = nc.dram_tensor("xg3", [MR, 128, G], MM_DT).ap()
    bxhd_d = nc.dram_tensor("bxhd", [1, G], MM_DT).ap()

    with tile.TileContext(nc) as tc:
        _body(tc, nc, v_d, catrep_d, iota3_d, sel8_d, e9_d, eye21_d, eye128_d,
              pw_d, wenc_d, benc_d, wx_d, bx_d, bh_d, wh_d, catemb_d, wdec_d,
              wddr_d, b3_d, out_d, xg3_d, bxhd_d)
    nc.compile()
    return nc


def _body(tc, nc, v_d, catrep_d, iota3_d, sel8_d, e9_d, eye21_d, eye128_d,
          pw_d, wenc_d, benc_d, wx_d, bx_d, bh_d, wh_d, catemb_d, wdec_d,
          wddr_d, b3_d, out_d, xg3_d, bxhd_d):
    import os
    PHASES = int(os.environ.get("KPHASES", "6"))
    with tc.tile_pool(name="persist", bufs=1) as persist:
        vT = persist.tile([128, KF, R], MM_DT)        # v transposed  [f%128, f//128, r]
        whsb = persist.tile([128, KH, G], MM_DT)      # Wh resident   [h%128, h//128, g]
        fuseT = persist.tile([128, KH, R], MM_DT)     # fuse^T        [h%128, h//128, r]
        pwT = persist.tile([128, KF, 22], MM_DT)      # [proto|Wgate]^T
        hT = persist.tile([128, KH, BC], MM_DT)       # h^T (LSTM state)
        cst = persist.tile([BC, H], F32)            # c (LSTM state)
        e9 = persist.tile([9, BC], MM_DT)             # [I8; ones]
        sel8 = persist.tile([128, BC], MM_DT)
        eye21 = persist.tile([21, 21], F32)
        eye128 = persist.tile([128, 128], F32)
        ones1 = persist.tile([1, 128], MM_DT)
        ones128 = persist.tile([128, 1], MM_DT)
        benc = persist.tile([128, KH], F32)
        wdec = persist.tile([128, KH, 2], MM_DT)
        wddr = persist.tile([128, MR, P2], F32)
        catemb = persist.tile([3, H], MM_DT)
        onehot = persist.tile([3, R], MM_DT)
        catb = persist.tile([3, R], F32)
        iota3 = persist.tile([3, 1], F32)
        bxh = persist.tile([1, G], MM_DT)
        pprow = persist.tile([1, 22], F32)
        ones1f = persist.tile([1, 128], F32)
        vv = persist.tile([128, MR], F32)
        vvp1 = persist.tile([128, MR], F32)
        vve = persist.tile([128, MR], F32)
        grs = persist.tile([128, MR, 2], MM_DT)
        b3bc = persist.tile([BC, 3], F32)
        osb = persist.tile([BC, 1], F32)
        gsb = persist.tile([BC, 1], F32)
        dsb = persist.tile([BC, 1], F32)
        fin = persist.tile([BC, 1], F32)

        # ---- const / weight loads ----
        nc.sync.dma_start(e9, e9_d)
        nc.sync.dma_start(sel8, sel8_d)
        nc.sync.dma_start(eye21, eye21_d)
        nc.sync.dma_start(eye128, eye128_d)
        nc.sync.dma_start(benc, benc_d)
        nc.sync.dma_start(wddr, wddr_d)
        nc.sync.dma_start(catemb, catemb_d)
        nc.sync.dma_start(iota3, iota3_d)
        onesf = persist.tile([128, 2], F32)
        nc.vector.memset(onesf, 1.0)
        nc.vector.tensor_copy(ones1, onesf[0:1, 0:1].to_broadcast((1, 128)))
        nc.vector.tensor_copy(ones1f, onesf[0:1, 0:1].to_broadcast((1, 128)))
        nc.vector.tensor_copy(ones128, onesf[:, 0:1])
        nc.vector.memset(wdec.bitcast(DT.uint32), 0)
        for k in range(KH):
            nc.sync.dma_start(wdec[:, k, 0:1],
                              wdec_d[k:k + 1, :].rearrange("a b -> b a"))
        nc.sync.dma_start(whsb, wh_d.rearrange("(ko p) g -> p ko g", p=128))
        # category one-hot: [3, R] rows of repeated category vs iota
        for i in range(3):
            nc.sync.dma_start(catb[i:i + 1, :], catrep_d)
        nc.vector.tensor_tensor(onehot, catb,
                                iota3[:, 0:1].to_broadcast((3, R)), ALU.is_equal)
        # bx + bh -> bxh, and park a copy in DRAM for per-step reload
        with tc.tile_pool(name="btmp", bufs=1) as btmp:
            bxs = btmp.tile([1, G], F32)
            bhs = btmp.tile([1, G], F32)
            nc.sync.dma_start(bxs, bx_d)
            nc.sync.dma_start(bhs, bh_d)
            nc.vector.tensor_add(bxh, bxs, bhs)
            nc.sync.dma_start(bxhd_d, bxh)

        if PHASES < 1:
            return
        # ---- phase 1: v load, PE-transpose into vT, row norms vv ----
        with tc.tile_pool(name="vchunk", bufs=2) as vchp, \
             tc.tile_pool(name="pst", bufs=4, space="PSUM") as pst, \
             tc.tile_pool(name="psvv", bufs=2) as psvv:
            for m in range(MR):
                vt = vchp.tile([128, F], F32)
                nc.sync.dma_start(vt, v_d[m * 128:(m + 1) * 128, :])
                if not os.environ.get("KSKIP_VV"):
                    sq = psvv.tile([128, F], F32)
                    nc.scalar.activation(sq, vt, AF.Square,
                                         accum_out=vv[:, m:m + 1])
                for ko in range(KF):
                    if os.environ.get("KSKIP_TR"):
                        break
                    pt = pst.tile([128, 128], F32)
                    nc.tensor.transpose(pt, vt[:, ko * 128:(ko + 1) * 128], eye128)
                    nc.vector.tensor_copy(vT[:, ko, m * 128:(m + 1) * 128], pt)
            if not os.environ.get("KSKIP_VV"):
                nc.vector.tensor_scalar_add(vvp1, vv, 1.0)
                nc.vector.tensor_scalar_add(vve, vv, 1e-8)

        if PHASES < 2:
            return
        # ---- phase 2: encoder GEMM -> fuseT = relu(Wenc^T vT + benc) + catsel ----
        with tc.tile_pool(name="wencs", bufs=4) as wencp, \
             tc.tile_pool(name="psf", bufs=2, space="PSUM") as psf, \
             tc.tile_pool(name="psc", bufs=2, space="PSUM") as psc:
            for m in range(KH):
                ps = psf.tile([128, R], F32)
                for ko in range(KF):
                    wt = wencp.tile([128, 128], MM_DT)
                    nc.sync.dma_start(
                        wt, wenc_d[ko * 128:(ko + 1) * 128,
                                   m * 128:(m + 1) * 128])
                    nc.tensor.matmul(ps, _mm(wt), _mm(vT[:, ko, :]),
                                     start=(ko == 0), stop=(ko == KF - 1))
                nc.scalar.activation(fuseT[:, m, :], ps, AF.Relu,
                                     bias=benc[:, m:m + 1])
                pc = psc.tile([128, R], F32)
                nc.tensor.matmul(pc, _mm(catemb[:, m * 128:(m + 1) * 128]),
                                 _mm(onehot), start=True, stop=True)
                nc.vector.tensor_add(fuseT[:, m, :], fuseT[:, m, :], pc)

        if PHASES < 3:
            return
        # ---- phase 3: xg GEMM -> xg3 DRAM (rows r = s*8+b, tiled [m,128,G]) ----
        with tc.tile_pool(name="wxs", bufs=8) as wxp, \
             tc.tile_pool(name="stage", bufs=3) as stagep, \
             tc.tile_pool(name="psx", bufs=4, space="PSUM") as psx:
            for nb in range(4):
                wts = []
                for k in range(KH):
                    wt = wxp.tile([128, 512], MM_DT)
                    nc.sync.dma_start(
                        wt, wx_d[k * 128:(k + 1) * 128,
                                 nb * 512:(nb + 1) * 512])
                    wts.append(wt)
                for m in range(MR):
                    ps = psx.tile([128, 512], F32)
                    for k in range(KH):
                        nc.tensor.matmul(
                            ps, _mm(fuseT[:, k, m * 128:(m + 1) * 128]),
                            _mm(wts[k]), start=(k == 0), stop=(k == KH - 1))
                    st = stagep.tile([128, 512], MM_DT)
                    nc.vector.tensor_copy(st, ps)
                    nc.sync.dma_start(
                        xg3_d[m, :, nb * 512:(nb + 1) * 512], st)

        if PHASES < 4:
            return
        # ---- phase 4: prototype prep: pwT, pp row ----
        with tc.tile_pool(name="prp", bufs=1) as prp, \
             tc.tile_pool(name="psp", bufs=2, space="PSUM") as psp:
            nc.vector.memset(pwT.bitcast(DT.uint32), 0)
            pwnat = prp.tile([21, F], F32)
            nc.sync.dma_start(pwnat, pw_d)
            for ko in range(KF):
                pt = psp.tile([128, 21], F32)
                nc.tensor.transpose(pt, pwnat[:, ko * 128:(ko + 1) * 128], eye21)
                nc.vector.tensor_copy(pwT[:, ko, 0:21], pt)
            pwsq = prp.tile([128, KF, 22], MM_DT)
            nc.vector.tensor_mul(pwsq, pwT, pwT)
            ppp = psp.tile([1, 22], F32)
            for ko in range(KF):
                nc.tensor.matmul(ppp, _mm(ones128), _mm(pwsq[:, ko, :]),
                                 start=(ko == 0), stop=(ko == KF - 1))
            nc.vector.tensor_copy(pprow, ppp)
            # scale proto cols by -2 (leave W_gate col 20 alone)
            nc.vector.tensor_scalar_mul(pwT[:, :, 0:P2], pwT[:, :, 0:P2], -2.0)

        if PHASES < 5:
            return
        # ---- phase 5: LSTM over S steps + decoder ----
        with tc.tile_pool(name="psl", bufs=4, space="PSUM") as psl, \
             tc.tile_pool(name="pstr", bufs=2, space="PSUM") as pstr, \
             tc.tile_pool(name="xgs", bufs=3) as xgsp, \
             tc.tile_pool(name="ga", bufs=8) as gap, \
             tc.tile_pool(name="hh", bufs=2) as hp, \
             tc.tile_pool(name="ltmp", bufs=2) as ltmp:
            for i in range(BC):
                nc.sync.dma_start(b3bc[i:i + 1, :], b3_d)
            hz = ltmp.tile([128, KH * BC], F32, tag="hz")
            nc.vector.memset(hz, 0.0)
            nc.vector.tensor_copy(hT, hz.rearrange("p (a b) -> p a b", a=KH))
            nc.vector.memset(cst, 0.0)
            h = None
            for s in range(S):
                xgs = xgsp.tile([9, G], MM_DT)
                off = (s * BC) % 128
                nc.sync.dma_start(xgs[0:BC, :], xg3_d[s // 16, off:off + BC, :])
                nc.sync.dma_start(xgs[BC:9, :], bxhd_d)
                ga = []
                for nb in range(4):
                    ps = psl.tile([BC, 512], F32, tag='l')
                    for k in range(KH):
                        nc.tensor.matmul(
                            ps, _mm(hT[:, k, :]),
                            _mm(whsb[:, k, nb * 512:(nb + 1) * 512]),
                            start=(k == 0), stop=False)
                    nc.tensor.matmul(ps, _mm(e9),
                                     _mm(xgs[:, nb * 512:(nb + 1) * 512]),
                                     start=False, stop=True)
                    g = gap.tile([BC, 512], F32)
                    nc.scalar.activation(g, ps,
                                         AF.Tanh if nb == 3 else AF.Sigmoid)
                    ga.append(g)
                t1 = ltmp.tile([BC, 512], F32)
                nc.vector.tensor_mul(t1, ga[0], ga[3])       # i*g
                nc.vector.tensor_mul(cst, cst, ga[1])        # f*c
                nc.vector.tensor_add(cst, cst, t1)           # c = f*c + i*g
                h = hp.tile([BC, H], F32)
                nc.vector.tensor_mul(h, ga[2], cst)          # h = o*c
                for k in range(KH):
                    pt = pstr.tile([128, BC], F32, tag='tr')
                    nc.tensor.transpose(pt, h[:, k * 128:(k + 1) * 128],
                                        eye128[0:BC, 0:BC])
                    nc.vector.tensor_copy(hT[:, k, :], pt)
            # decoder: sigmoid(h @ Wdec + bdec)
            pd = psl.tile([BC, 2], F32, tag='l')
            for k in range(KH):
                nc.tensor.matmul(pd, _mm(hT[:, k, :]), _mm(wdec[:, k, :]),
                                 start=(k == 0), stop=(k == KH - 1))
            nc.scalar.activation(osb, pd[:, 0:1], AF.Sigmoid, bias=b3bc[:, 0:1])

        if PHASES < 6:
            return
        # ---- phase 6: distances, gate, combine ----
        with tc.tile_pool(name="psq", bufs=4, space="PSUM") as psq, \
             tc.tile_pool(name="dfp", bufs=2) as dfp:
            for m in range(MR):
                pp_ = psq.tile([128, 22], F32, tag='q')
                for ko in range(KF):
                    nc.tensor.matmul(pp_, _mm(vT[:, ko, m * 128:(m + 1) * 128]),
                                     _mm(pwT[:, ko, :]),
                                     start=(ko == 0), stop=False)
                nc.tensor.matmul(pp_[:, 0:P2], ones1f,
                                 pprow[:, 0:P2], start=False, stop=True)
                a_ = dfp.tile([128, P2], F32)
                b_ = dfp.tile([128, P2], F32)
                nc.scalar.activation(a_, pp_[:, 0:P2], AF.Ln,
                                     bias=vvp1[:, m:m + 1])
                nc.scalar.activation(b_, pp_[:, 0:P2], AF.Ln,
                                     bias=vve[:, m:m + 1])
                df = dfp.tile([128, P2], F32)
                nc.vector.tensor_sub(df, a_, b_)
                nc.vector.tensor_mul(df, df, wddr[:, m, :])
                with nc.allow_low_precision(reason="20-elem reduce into f32r"):
                    nc.vector.reduce_sum(out=grs[:, m, 1:2], in_=df, axis=AX.X)
                nc.scalar.copy(grs[:, m, 0:1], pp_[:, P2:P2 + 1])
            pr = psq.tile([BC, 2], F32, tag='q')
            for m in range(MR):
                nc.tensor.matmul(pr, _mm(sel8), _mm(grs[:, m, :]),
                                 start=(m == 0), stop=(m == MR - 1))
            nc.scalar.activation(gsb, pr[:, 0:1], AF.Sigmoid,
                                 bias=b3bc[:, 1:2], scale=1.0 / S)
            nc.scalar.activation(dsb, pr[:, 1:2], AF.Sigmoid,
                                 bias=b3bc[:, 2:3])
            nc.vector.tensor_sub(fin, osb, dsb)
            nc.vector.tensor_mul(fin, fin, gsb)
            nc.vector.tensor_add(fin, fin, dsb)
            nc.sync.dma_start(out_d, fin)


_NC_CACHE = {}


def _get_nc():
    if "nc" not in _NC_CACHE:
        _NC_CACHE["nc"] = build_nc()
    return _NC_CACHE["nc"]


def _make_in_maps(v_feat, category, W_enc, b_enc, Wx, bx, Wh, bh, cat_emb,
                  W_dec, b_dec, prototype, W_dd, b_dd, W_gate, b_gate):
    f32 = np.float32
    v_feat = np.ascontiguousarray(np.asarray(v_feat, f32))
    category = np.asarray(category).astype(np.int64)
    eye128 = np.eye(128, dtype=f32)
    eye21 = np.eye(21, dtype=f32)
    e9 = np.concatenate([np.eye(BC, dtype=f32),
                         np.ones((1, BC), f32)], axis=0)
    sel8 = np.zeros((128, BC), f32)
    sel8[np.arange(128), np.arange(128) % BC] = 1.0
    iota3 = np.arange(3, dtype=f32).reshape(3, 1)
    pw = np.concatenate([np.asarray(prototype, f32),
                         np.asarray(W_gate, f32).reshape(1, F)], axis=0)
    benc = np.asarray(b_enc, f32).reshape(KH, 128).T.copy()
    wdec = np.asarray(W_dec, f32).reshape(KH, 128)
    wdd = np.asarray(W_dd, f32).reshape(S, P2)
    rep = np.repeat(wdd[:, None, :], BC, axis=1).reshape(R, P2)
    wddrep = np.ascontiguousarray(
        rep.reshape(MR, 128, P2).transpose(1, 0, 2))
    b3 = np.array([[np.asarray(b_dec, f32).reshape(-1)[0],
                    np.asarray(b_gate, f32).reshape(-1)[0],
                    np.asarray(b_dd, f32).reshape(-1)[0]]], f32)
    common = {
        "iota3": iota3, "sel8": sel8, "e9": e9, "eye21": eye21,
        "eye128": eye128, "pw": pw,
        "wenc": np.ascontiguousarray(np.asarray(W_enc, f32)),
        "benc": benc,
        "wx": np.ascontiguousarray(np.asarray(Wx, f32)),
        "bx": np.asarray(bx, f32).reshape(1, G),
        "bh": np.asarray(bh, f32).reshape(1, G),
        "wh": np.ascontiguousarray(np.asarray(Wh, f32)),
        "catemb": np.ascontiguousarray(np.asarray(cat_emb, f32)),
        "wdec": wdec, "wddrep": wddrep, "b3": b3,
    }
    in_maps = []
    for j in range(NCORES):
        vs = np.ascontiguousarray(
            v_feat[j * BC:(j + 1) * BC].transpose(1, 0, 2).reshape(R, F))
        catrep = np.tile(category[j * BC:(j + 1) * BC].astype(f32),
                         S).reshape(1, R)
        in_maps.append({"v": vs, "catrep": catrep, **common})
    return in_maps


def run(trace=False, **inputs):
    nc = _get_nc()
    in_maps = _make_in_maps(**inputs)
    res = run_bass_kernel_spmd(nc, in_maps, list(range(NCORES)), trace=trace)
    out = np.concatenate([res.results[j]["out"] for j in range(NCORES)],
                         axis=0).astype(np.float32)
    return out, res


def kernel(**inputs):
    out, _ = run(trace=False, **inputs)
    return out



# revision 8
# speedup vs baseline: 2.1592x; 2.1592x over previous
"""Trainium2 Bass kernel for nn_ASD_RNN (encoder + fused-gate LSTM + prototype-distance head).

Contract: kernel(**inputs) takes FULL unsharded inputs (as in reference.setup_inputs())
and returns the FULL [64, 1] float32 output. Internally shards batch across 8 cores
(8 batches per core), runs one Bass kernel per core via run_bass_kernel_spmd, gathers.

Per-core layout (BC = 8 batches, R = BC*S = 512 rows, s-major: r = s*8 + b):
  - All GEMM operands are bf16; PSUM accumulation is fp32.
  - v is shipped pre-transposed from the host: vT [f%128, f//128, r] feeds the
    encoder and prototype-distance GEMMs; a row-major copy vrow feeds ||v||^2.
  - Encoder: fuseT[h%128, h//128, r] = relu(Wenc^T v + benc) + cat_emb one-hot fold.
  - xg = fuse @ Wx + (bx+bh), computed row-major into SBUF-resident xgsb
    [r%128, r//128, g] (bias added via a host-replicated [128, G] tensor).
  - LSTM keeps h transposed (hT [h%128, k, b]) as the matmul stationary; Wh is the
    moving operand (one full Wh pass per step is the PE floor). The per-step xg+bias
    contribution is folded into the gates PSUM with a 32-row selector matmul
    (eyevar) so no per-step DMA or staging copies are needed.
  - Distance head: ||v-p||^2 via matmul folds (-2p | W_gate rides as column 20),
    log-ratio via two Ln activations with per-partition bias, reduced via a
    selector matmul.
"""

import numpy as np
import ml_dtypes

import concourse.bass as bass
import concourse.mybir as mybir
import concourse.tile as tile
from concourse import bacc
from concourse.bass_utils import run_bass_kernel_spmd

AF = mybir.ActivationFunctionType
ALU = mybir.AluOpType
DT = mybir.dt
AX = mybir.AxisListType

B, S, F, H, P2 = 64, 64, 2048, 512, 20
G = 4 * H
NCORES = 8
BC = B // NCORES          # 8 batches per core
R = BC * S                # 512 rows per core
KF = F // 128             # 16 k-tiles over F
KH = H // 128             # 4 k-tiles over H
MR = R // 128             # 4 row tiles
F32 = DT.float32
BF = DT.bfloat16
MMF = DT.float32r
BF_NP = ml_dtypes.bfloat16


def build_nc():
    nc = bacc.Bacc("TRN2", target_bir_lowering=False, debug=False,
                   num_devices=NCORES)

    def din(name, shape, dt=BF):
        return nc.dram_tensor(name, shape, dt, kind="ExternalInput").ap()

    vT_d = din("vT", [128, KF, R])
    vrow_d = din("vrow", [128, MR, F])
    wenc_d = din("wencp", [128, KF, H])
    benc_d = din("benc", [128, KH], F32)
    catemb_d = din("catemb", [3, H])
    onehot_d = din("onehot", [3, R])
    wx_d = din("wxp", [128, KH, G])
    bxh_d = din("bxh128", [128, G], F32)
    wh_d = din("whp", [128, KH, G])
    eyevar_d = din("eyevar", [128, 8, BC])
    eye8_d = din("eye8", [BC, BC])
    pwT_d = din("pwT", [128, KF, 22])
    pprow_d = din("pprow", [1, 22], MMF)
    ones1_d = din("ones1", [1, 128], MMF)
    wddr_d = din("wddr", [128, MR, P2], F32)
    sel8_d = din("sel8", [128, BC], MMF)
    wdec_d = din("wdecp", [128, KH])
    b3bc_d = din("b3bc", [BC, 3], F32)
    out_d = nc.dram_tensor("out", [BC, 1], F32, kind="ExternalOutput").ap()

    with tile.TileContext(nc) as tc:
        _body(tc, nc, vT_d, vrow_d, wenc_d, benc_d, catemb_d, onehot_d, wx_d,
              bxh_d, wh_d, eyevar_d, eye8_d, pwT_d, pprow_d, ones1_d, wddr_d,
              sel8_d, wdec_d, b3bc_d, out_d)
    nc.compile()
    return nc


def _body(tc, nc, vT_d, vrow_d, wenc_d, benc_d, catemb_d, onehot_d, wx_d,
          bxh_d, wh_d, eyevar_d, eye8_d, pwT_d, pprow_d, ones1_d, wddr_d,
          sel8_d, wdec_d, b3bc_d, out_d):
    import os
    PHASES = int(os.environ.get("KPHASES", "9"))
    with tc.tile_pool(name="persist", bufs=1) as P:
        vT = P.tile([128, KF, R], BF)
        wencp = P.tile([128, KF, H], BF)
        wxp = P.tile([128, KH, G], BF)
        whsb = P.tile([128, KH, G], BF)
        fuseT = P.tile([128, KH, R], BF)
        xgsb = P.tile([128, MR, G], BF)
        bxh128 = P.tile([128, G], F32)
        catemb = P.tile([3, H], BF)
        onehot = P.tile([3, R], BF)
        benc = P.tile([128, KH], F32)
        pwT = P.tile([128, KF, 22], BF)
        pprow = P.tile([1, 22], MMF)
        ones1f = P.tile([1, 128], MMF)
        wddr = P.tile([128, MR, P2], F32)
        sel8 = P.tile([128, BC], MMF)
        eyevar = P.tile([128, 8, BC], BF)
        eye8 = P.tile([BC, BC], BF)
        wdecp = P.tile([128, KH], BF)
        b3bc = P.tile([BC, 3], F32)
        hT = P.tile([128, KH, BC], BF)
        cst = P.tile([BC, H], BF)
        vv = P.tile([128, MR], F32)
        vvp1 = P.tile([128, MR], F32)
        vve = P.tile([128, MR], F32)
        grs = P.tile([128, MR, 2], MMF)
        osb = P.tile([BC, 1], F32)
        gsb = P.tile([BC, 1], F32)
        dsb = P.tile([BC, 1], F32)
        fin = P.tile([BC, 1], F32)

        # ---- input DMAs ----
        nc.sync.dma_start(vT, vT_d)
        nc.sync.dma_start(wencp, wenc_d)
        nc.sync.dma_start(wxp, wx_d)
        nc.sync.dma_start(whsb, wh_d)
        nc.sync.dma_start(bxh128, bxh_d)
        nc.sync.dma_start(catemb, catemb_d)
        nc.sync.dma_start(onehot, onehot_d)
        nc.sync.dma_start(benc, benc_d)
        nc.sync.dma_start(pwT, pwT_d)
        nc.sync.dma_start(pprow, pprow_d)
        nc.sync.dma_start(ones1f, ones1_d)
        nc.sync.dma_start(wddr, wddr_d)
        nc.sync.dma_start(sel8, sel8_d)
        nc.sync.dma_start(eyevar, eyevar_d)
        nc.sync.dma_start(eye8, eye8_d)
        nc.sync.dma_start(wdecp, wdec_d)
        nc.sync.dma_start(b3bc, b3bc_d)

        # ---- vv[r] = sum_f v[r,f]^2 (Act engine, overlapped with encoder) ----
        if PHASES >= 1:
            with tc.tile_pool(name="vvp", bufs=1) as VP, \
                 tc.tile_pool(name="vsq", bufs=2) as SQ:
                vrow = VP.tile([128, MR, F], BF)
                nc.sync.dma_start(vrow, vrow_d)
                for m in range(MR):
                    sq = SQ.tile([128, F], BF)
                    nc.scalar.activation(sq, vrow[:, m, :], AF.Square,
                                         accum_out=vv[:, m:m + 1])
                nc.vector.tensor_scalar_add(vvp1, vv, 1.0)
                nc.vector.tensor_scalar_add(vve, vv, 1e-8)

        # ---- encoder: fuseT = relu(Wenc^T v + benc) + catemb-fold ----
        if PHASES >= 2:
            with tc.tile_pool(name="psf", bufs=2, space="PSUM") as psf, \
                 tc.tile_pool(name="psc", bufs=2, space="PSUM") as psc, \
                 tc.tile_pool(name="encs", bufs=2) as encs:
                for m in range(KH):
                    ps = psf.tile([128, R], F32)
                    for ko in range(KF):
                        nc.tensor.matmul(
                            ps, wencp[:, ko, m * 128:(m + 1) * 128],
                            vT[:, ko, :], start=(ko == 0), stop=(ko == KF - 1))
                    pc = psc.tile([128, R], F32)
                    nc.tensor.matmul(pc, catemb[:, m * 128:(m + 1) * 128],
                                     onehot, start=True, stop=True)
                    sc = encs.tile([128, R], F32)
                    nc.scalar.activation(sc, ps, AF.Relu,
                                         bias=benc[:, m:m + 1])
                    nc.vector.tensor_add(fuseT[:, m, :], sc, pc)

        # ---- xg = fuse @ Wx + (bx+bh), row-major into SBUF ----
        if PHASES >= 3:
            with tc.tile_pool(name="psx", bufs=4, space="PSUM") as psx:
                for nb in range(4):
                    for m in range(MR):
                        ps = psx.tile([128, 512], F32)
                        for k in range(KH):
                            nc.tensor.matmul(
                                ps, fuseT[:, k, m * 128:(m + 1) * 128],
                                wxp[:, k, nb * 512:(nb + 1) * 512],
                                start=(k == 0), stop=(k == KH - 1))
                        nc.vector.scalar_tensor_tensor(
                            xgsb[:, m, nb * 512:(nb + 1) * 512], ps, 1.0,
                            bxh128[:, nb * 512:(nb + 1) * 512],
                            ALU.mult, ALU.add)

        # ---- prototype distances + gate head ----
        if PHASES >= 4:
            with tc.tile_pool(name="psq", bufs=2, space="PSUM") as psq, \
                 tc.tile_pool(name="psr", bufs=1, space="PSUM") as psr, \
                 tc.tile_pool(name="dfp", bufs=4) as dfp:
                for m in range(MR):
                    pq = psq.tile([128, 22], F32)
                    for ko in range(KF):
                        nc.tensor.matmul(pq, vT[:, ko, m * 128:(m + 1) * 128],
                                         pwT[:, ko, :],
                                         start=(ko == 0), stop=False)
                    nc.tensor.matmul(pq, ones1f, pprow,
                                     start=False, stop=True)
                    a_ = dfp.tile([128, P2], F32)
                    b_ = dfp.tile([128, P2], F32)
                    nc.scalar.activation(a_, pq[:, 0:P2], AF.Ln,
                                         bias=vvp1[:, m:m + 1])
                    nc.scalar.activation(b_, pq[:, 0:P2], AF.Ln,
                                         bias=vve[:, m:m + 1])
                    df = dfp.tile([128, P2], F32)
                    nc.vector.tensor_sub(df, a_, b_)
                    nc.vector.tensor_mul(df, df, wddr[:, m, :])
                    with nc.allow_low_precision(reason="20-elem reduce to f32r"):
                        nc.vector.reduce_sum(out=grs[:, m, 1:2], in_=df,
                                             axis=AX.X)
                    nc.scalar.copy(grs[:, m, 0:1], pq[:, P2:P2 + 1])
                pr = psr.tile([BC, 2], F32)
                for m in range(MR):
                    nc.tensor.matmul(pr, sel8, grs[:, m, :],
                                     start=(m == 0), stop=(m == MR - 1))
                nc.scalar.activation(gsb, pr[:, 0:1], AF.Sigmoid,
                                     bias=b3bc[:, 1:2], scale=1.0 / S)
                nc.scalar.activation(dsb, pr[:, 1:2], AF.Sigmoid,
                                     bias=b3bc[:, 2:3])

        # ---- LSTM over S steps + decoder ----
        if PHASES >= 5:
            NBORD = (0, 3, 1, 2)  # i, g, f, o: lets c-update overlap o-gates
            with tc.tile_pool(name="psl", bufs=4, space="PSUM") as psl, \
                 tc.tile_pool(name="pstr", bufs=3, space="PSUM") as pstr, \
                 tc.tile_pool(name="psd", bufs=1, space="PSUM") as psd, \
                 tc.tile_pool(name="gap", bufs=8) as gap, \
                 tc.tile_pool(name="hp", bufs=2) as hp, \
                 tc.tile_pool(name="ltp", bufs=2) as ltp:
                for s in range(S):
                    m = s // 16
                    p0 = (s * 8) % 128
                    blk = (p0 // 64) * 64
                    q = (p0 % 64) // 8
                    # xg+bias fold first: independent of h, fills the PE gap
                    # left by the previous step's act/DVE tail.
                    pss = []
                    for nb in NBORD:
                        ps = psl.tile([BC, 512], F32, tag="l")
                        nc.tensor.matmul(
                            ps, eyevar[blk:blk + 64, q, :],
                            xgsb[blk:blk + 64, m, nb * 512:(nb + 1) * 512],
                            start=True, stop=(s == 0))
                        pss.append(ps)
                    ga = {}
                    for ps, nb in zip(pss, NBORD):
                        if s > 0:
                            for k in range(KH):
                                nc.tensor.matmul(
                                    ps, hT[:, k, :],
                                    whsb[:, k, nb * 512:(nb + 1) * 512],
                                    start=False, stop=(k == KH - 1))
                        g = gap.tile([BC, 512], BF)
                        nc.scalar.activation(g, ps,
                                             AF.Tanh if nb == 3 else AF.Sigmoid)
                        ga[nb] = g
                    if s == 0:
                        nc.vector.tensor_mul(cst, ga[0], ga[3])
                    else:
                        t1 = ltp.tile([BC, H], BF)
                        nc.vector.tensor_mul(t1, ga[0], ga[3])   # i*g
                        nc.vector.tensor_mul(cst, cst, ga[1])    # f*c
                        nc.vector.tensor_add(cst, cst, t1)
                    h = hp.tile([BC, H], BF)
                    nc.vector.tensor_mul(h, ga[2], cst)          # h = o*c
                    for k in range(KH):
                        pt = pstr.tile([128, BC], BF, tag="tr")
                        nc.tensor.transpose(pt, h[:, k * 128:(k + 1) * 128],
                                            eye8)
                        nc.vector.tensor_copy(hT[:, k, :], pt)
                # decoder
                pd = psd.tile([BC, 1], F32)
                for k in range(KH):
                    nc.tensor.matmul(pd, hT[:, k, :], wdecp[:, k:k + 1],
                                     start=(k == 0), stop=(k == KH - 1))
                nc.scalar.activation(osb, pd, AF.Sigmoid, bias=b3bc[:, 0:1])

        # ---- combine ----
        if PHASES >= 6:
            nc.vector.tensor_sub(fin, osb, dsb)
            nc.vector.scalar_tensor_tensor(fin, fin, gsb[:, 0:1], dsb,
                                           ALU.mult, ALU.add)
            nc.sync.dma_start(out_d, fin)


_NC_CACHE = {}


def _get_nc():
    if "nc" not in _NC_CACHE:
        _NC_CACHE["nc"] = build_nc()
    return _NC_CACHE["nc"]


def _make_in_maps(v_feat, category, W_enc, b_enc, Wx, bx, Wh, bh, cat_emb,
                  W_dec, b_dec, prototype, W_dd, b_dd, W_gate, b_gate):
    f32 = np.float32
    v_feat = np.asarray(v_feat, f32)
    category = np.asarray(category).astype(np.int64)

    wencp = np.ascontiguousarray(
        np.asarray(W_enc, f32).reshape(KF, 128, H).transpose(1, 0, 2)
    ).astype(BF_NP)
    benc = np.ascontiguousarray(
        np.asarray(b_enc, f32).reshape(KH, 128).T).copy()
    catemb = np.asarray(cat_emb, f32).astype(BF_NP)
    wxp = np.ascontiguousarray(
        np.asarray(Wx, f32).reshape(KH, 128, G).transpose(1, 0, 2)
    ).astype(BF_NP)
    bxh128 = np.ascontiguousarray(
        np.tile((np.asarray(bx, f32) + np.asarray(bh, f32)).reshape(1, G),
                (128, 1)))
    whp = np.ascontiguousarray(
        np.asarray(Wh, f32).reshape(KH, 128, G).transpose(1, 0, 2)
    ).astype(BF_NP)
    # eyevar[p, q, j] = 1 iff p%64 == q*8+j  (64-aligned step-row selector)
    pp_ = np.arange(128)
    eyevar = np.zeros((128, 8, BC), f32)
    for qq in range(8):
        for j in range(BC):
            eyevar[pp_ % 64 == qq * 8 + j, qq, j] = 1.0
    eyevar = eyevar.astype(BF_NP)
    eye8 = np.eye(BC, dtype=f32).astype(BF_NP)
    proto = np.asarray(prototype, f32)
    pw = np.concatenate([-2.0 * proto,
                         np.asarray(W_gate, f32).reshape(1, F),
                         np.zeros((1, F), f32)], axis=0)  # [22, F]
    pwT = np.ascontiguousarray(
        pw.T.reshape(KF, 128, 22).transpose(1, 0, 2)).astype(BF_NP)
    pprow = np.concatenate([(proto * proto).sum(axis=1),
                            np.zeros(2, f32)]).reshape(1, 22).astype(f32)
    ones1 = np.ones((1, 128), f32)
    wdd = np.asarray(W_dd, f32).reshape(S, P2)
    rep = np.repeat(wdd[:, None, :], BC, axis=1).reshape(R, P2)
    wddr = np.ascontiguousarray(rep.reshape(MR, 128, P2).transpose(1, 0, 2))
    sel8 = np.zeros((128, BC), f32)
    sel8[np.arange(128), np.arange(128) % BC] = 1.0
    wdecp = np.ascontiguousarray(
        np.asarray(W_dec, f32).reshape(KH, 128).T).astype(BF_NP)
    b3 = np.array([np.asarray(b_dec, f32).reshape(-1)[0],
                   np.asarray(b_gate, f32).reshape(-1)[0],
                   np.asarray(b_dd, f32).reshape(-1)[0]], f32)
    b3bc = np.ascontiguousarray(np.tile(b3.reshape(1, 3), (BC, 1)))

    common = {
        "wencp": wencp, "benc": benc, "catemb": catemb, "wxp": wxp,
        "bxh128": bxh128, "whp": whp, "eyevar": eyevar, "eye8": eye8,
        "pwT": pwT, "pprow": pprow, "ones1": ones1, "wddr": wddr,
        "sel8": sel8, "wdecp": wdecp, "b3bc": b3bc,
    }
    in_maps = []
    for j in range(NCORES):
        vs = np.ascontiguousarray(
            v_feat[j * BC:(j + 1) * BC].transpose(1, 0, 2).reshape(R, F))
        vTn = np.ascontiguousarray(
            vs.reshape(R, KF, 128).transpose(2, 1, 0)).astype(BF_NP)
        vrow = np.ascontiguousarray(
            vs.reshape(MR, 128, F).transpose(1, 0, 2)).astype(BF_NP)
        cats = category[j * BC:(j + 1) * BC]
        onehot = (cats[None, :] == np.arange(3)[:, None]).astype(f32)
        onehot = np.ascontiguousarray(
            np.tile(onehot, (1, S))).astype(BF_NP)  # [3, R], r = s*8+b
        in_maps.append({"vT": vTn, "vrow": vrow, "onehot": onehot, **common})
    return in_maps


def run(trace=False, **inputs):
    nc = _get_nc()
    in_maps = _make_in_maps(**inputs)
    res = run_bass_kernel_spmd(nc, in_maps, list(range(NCORES)), trace=trace)
    out = np.concatenate([res.results[j]["out"] for j in range(NCORES)],
                         axis=0).astype(np.float32)
    return out, res


def kernel(**inputs):
    out, _ = run(trace=False, **inputs)
    return out


# revision 31
# speedup vs baseline: 2.4118x; 1.1170x over previous
"""Trainium2 Bass kernel for nn_ASD_RNN (encoder + fused-gate LSTM + prototype-distance head).

Contract: kernel(**inputs) takes FULL unsharded inputs (as in reference.setup_inputs())
and returns the FULL [64, 1] float32 output. Internally shards batch across 8 cores
(8 batches per core), runs one Bass kernel per core via run_bass_kernel_spmd, gathers.

Per-core layout (BC = 8 batches, R = BC*S = 512 rows, s-major: r = s*8 + b):
  - All GEMM operands are bf16; PSUM accumulation is fp32.
  - v is shipped pre-transposed from the host: vT [f%128, f//128, r] feeds the
    encoder and prototype-distance GEMMs; a row-major copy vrow feeds ||v||^2.
  - Encoder: fuseT[h%128, h//128, r] = relu(Wenc^T v + benc) + cat_emb one-hot fold.
  - xg = fuse @ Wx + (bx+bh), computed row-major into SBUF-resident xgsb
    [r%128, r//128, g] (bias added via a host-replicated [128, G] tensor).
  - LSTM keeps h transposed (hT [h%128, k, b]) as the matmul stationary; Wh is the
    moving operand (one full Wh pass per step is the PE floor). The per-step xg+bias
    contribution is folded into the gates PSUM with a 32-row selector matmul
    (eyevar) so no per-step DMA or staging copies are needed.
  - Distance head: ||v-p||^2 via matmul folds (-2p | W_gate rides as column 20),
    log-ratio via two Ln activations with per-partition bias, reduced via a
    selector matmul.
"""

import numpy as np
import ml_dtypes

import concourse.bass as bass
import concourse.mybir as mybir
import concourse.tile as tile
from concourse import bacc
from concourse.bass_utils import run_bass_kernel_spmd

AF = mybir.ActivationFunctionType
ALU = mybir.AluOpType
DT = mybir.dt
AX = mybir.AxisListType

B, S, F, H, P2 = 64, 64, 2048, 512, 20
G = 4 * H
NCORES = 8
BC = B // NCORES          # 8 batches per core
R = BC * S                # 512 rows per core
KF = F // 128             # 16 k-tiles over F
KH = H // 128             # 4 k-tiles over H
MR = R // 128             # 4 row tiles
F32 = DT.float32
BF = DT.bfloat16
FP8 = DT.float8e4
MMF = DT.float32r
BF_NP = ml_dtypes.bfloat16
FP8_NP = ml_dtypes.float8_e4m3
WH_SCALE = 32.0  # Wh/W_dec stored fp8 as x32; pre-acts carry x32, acts undo


def build_nc():
    nc = bacc.Bacc("TRN2", target_bir_lowering=False, debug=False,
                   num_devices=NCORES)

    def din(name, shape, dt=BF):
        return nc.dram_tensor(name, shape, dt, kind="ExternalInput").ap()

    vT_d = din("vT", [128, KF, R])
    vrow_d = din("vrow", [128, MR, F])
    wenc_d = din("wencp", [128, KF, H])
    benc_d = din("benc", [128, KH], F32)
    catemb_d = din("catemb", [3, H])
    onehot_d = din("onehot", [3, R])
    wx_d = din("wxp", [128, KH, G])
    bxh_d = din("bxh128", [128, G], F32)
    wh_d = din("whp8", [128, KH, G], FP8)
    eyevar_d = din("eyevar", [128, 8, 32])
    eye8_d = din("eye8", [BC, BC])
    pwT_d = din("pwT", [128, KF, 22])
    pprow_d = din("pprow", [1, 22], MMF)
    ones1_d = din("ones1", [1, 128], MMF)
    wddr_d = din("wddr", [128, MR, P2], F32)
    sel8_d = din("sel8", [128, BC], MMF)
    wdec_d = din("wdecp8", [128, KH], FP8)
    b3bc_d = din("b3bc", [BC, 3], F32)
    out_d = nc.dram_tensor("out", [BC, 1], F32, kind="ExternalOutput").ap()

    with tile.TileContext(nc) as tc:
        _body(tc, nc, vT_d, vrow_d, wenc_d, benc_d, catemb_d, onehot_d, wx_d,
              bxh_d, wh_d, eyevar_d, eye8_d, pwT_d, pprow_d, ones1_d, wddr_d,
              sel8_d, wdec_d, b3bc_d, out_d)
    nc.compile()
    return nc


def _body(tc, nc, vT_d, vrow_d, wenc_d, benc_d, catemb_d, onehot_d, wx_d,
          bxh_d, wh_d, eyevar_d, eye8_d, pwT_d, pprow_d, ones1_d, wddr_d,
          sel8_d, wdec_d, b3bc_d, out_d):
    import os
    PHASES = int(os.environ.get("KPHASES", "9"))
    with tc.tile_pool(name="persist", bufs=1) as P:
        vT = P.tile([128, KF, R], BF)
        wencp = P.tile([128, KF, H], BF)
        wxp = P.tile([128, KH, G], BF)
        whsb = P.tile([128, KH, G], FP8)
        fuseT = P.tile([128, KH, R], BF)
        xgsb = P.tile([128, MR, G], BF)
        bxh128 = P.tile([128, G], F32)
        catemb = P.tile([3, H], BF)
        onehot = P.tile([3, R], BF)
        benc = P.tile([128, KH], F32)
        pwT = P.tile([128, KF, 22], BF)
        pprow = P.tile([1, 22], MMF)
        ones1f = P.tile([1, 128], MMF)
        wddr = P.tile([128, MR, P2], F32)
        sel8 = P.tile([128, BC], MMF)
        eyevar = P.tile([128, 8, 32], BF)
        eye8 = P.tile([BC, BC], BF)
        wdecp = P.tile([128, KH], FP8)
        b3bc = P.tile([BC, 3], F32)
        # 32-wide out-partition padding: DoubleRow needs >=32 stationary cols
        hT = P.tile([128, KH, 32], FP8)
        cst = P.tile([BC, H], BF)
        vv = P.tile([128, MR], F32)
        vvp1 = P.tile([128, MR], F32)
        vve = P.tile([128, MR], F32)
        grs = P.tile([128, MR, 2], MMF)
        osb = P.tile([BC, 1], F32)
        gsb = P.tile([BC, 1], F32)
        dsb = P.tile([BC, 1], F32)
        fin = P.tile([BC, 1], F32)

        # ---- input DMAs (queue order = priority: encoder set first) ----
        nc.sync.dma_start(vT, vT_d)
        nc.sync.dma_start(wencp, wenc_d)
        nc.sync.dma_start(benc, benc_d)
        nc.sync.dma_start(catemb, catemb_d)
        nc.sync.dma_start(onehot, onehot_d)
        nc.sync.dma_start(wxp, wx_d)
        nc.sync.dma_start(bxh128, bxh_d)
        nc.sync.dma_start(whsb, wh_d)
        nc.sync.dma_start(eyevar, eyevar_d)
        nc.sync.dma_start(eye8, eye8_d)
        nc.sync.dma_start(pwT, pwT_d)
        nc.sync.dma_start(pprow, pprow_d)
        nc.sync.dma_start(ones1f, ones1_d)
        nc.sync.dma_start(wddr, wddr_d)
        nc.sync.dma_start(sel8, sel8_d)
        nc.sync.dma_start(wdecp, wdec_d)
        nc.sync.dma_start(b3bc, b3bc_d)

        # ---- encoder: fuseT = relu(Wenc^T v + benc) + catemb-fold ----
        if PHASES >= 2:
            with tc.tile_pool(name="psf", bufs=2, space="PSUM") as psf, \
                 tc.tile_pool(name="psc", bufs=2, space="PSUM") as psc, \
                 tc.tile_pool(name="encs", bufs=2) as encs:
                for m in range(KH):
                    ps = psf.tile([128, R], F32)
                    for ko in range(KF):
                        nc.tensor.matmul(
                            ps, wencp[:, ko, m * 128:(m + 1) * 128],
                            vT[:, ko, :], start=(ko == 0), stop=(ko == KF - 1))
                    pc = psc.tile([128, R], F32)
                    nc.tensor.matmul(pc, catemb[:, m * 128:(m + 1) * 128],
                                     onehot, start=True, stop=True)
                    sc = encs.tile([128, R], F32)
                    nc.scalar.activation(sc, ps, AF.Relu,
                                         bias=benc[:, m:m + 1])
                    nc.vector.tensor_add(fuseT[:, m, :], sc, pc)

        # ---- vv[r] = sum_f v[r,f]^2 (Act engine; emitted after the encoder
        # relus so a late vrow DMA can't stall them on the in-order engine) --
        if PHASES >= 1:
            with tc.tile_pool(name="vvp", bufs=1) as VP, \
                 tc.tile_pool(name="vsq", bufs=2) as SQ:
                vrow = VP.tile([128, MR, F], BF)
                nc.sync.dma_start(vrow, vrow_d)
                for m in range(MR):
                    sq = SQ.tile([128, F], BF)
                    nc.scalar.activation(sq, vrow[:, m, :], AF.Square,
                                         accum_out=vv[:, m:m + 1])
                nc.vector.tensor_scalar_add(vvp1, vv, 1.0)
                nc.vector.tensor_scalar_add(vve, vv, 1e-8)

        # ---- xg = fuse @ Wx + (bx+bh), row-major into SBUF ----
        if PHASES >= 3:
            with tc.tile_pool(name="psx", bufs=4, space="PSUM") as psx:
                for nb in range(4):
                    for m in range(MR):
                        ps = psx.tile([128, 512], F32)
                        for k in range(KH):
                            nc.tensor.matmul(
                                ps, fuseT[:, k, m * 128:(m + 1) * 128],
                                wxp[:, k, nb * 512:(nb + 1) * 512],
                                start=(k == 0), stop=(k == KH - 1))
                        # xg carries x32 so it can fold into the x32 fp8
                        # Wh partials; bxh128 is host-prescaled by 32.
                        nc.vector.scalar_tensor_tensor(
                            xgsb[:, m, nb * 512:(nb + 1) * 512], ps, WH_SCALE,
                            bxh128[:, nb * 512:(nb + 1) * 512],
                            ALU.mult, ALU.add)

        # ---- prototype distances + gate head ----
        if PHASES >= 4:
            with tc.tile_pool(name="psq", bufs=2, space="PSUM") as psq, \
                 tc.tile_pool(name="psr", bufs=1, space="PSUM") as psr, \
                 tc.tile_pool(name="dfp", bufs=4) as dfp:
                for m in range(MR):
                    pq = psq.tile([128, 22], F32)
                    for ko in range(KF):
                        nc.tensor.matmul(pq, vT[:, ko, m * 128:(m + 1) * 128],
                                         pwT[:, ko, :],
                                         start=(ko == 0), stop=False)
                    nc.tensor.matmul(pq, ones1f, pprow,
                                     start=False, stop=True)
                    a_ = dfp.tile([128, P2], F32)
                    b_ = dfp.tile([128, P2], F32)
                    nc.scalar.activation(a_, pq[:, 0:P2], AF.Ln,
                                         bias=vvp1[:, m:m + 1])
                    nc.scalar.activation(b_, pq[:, 0:P2], AF.Ln,
                                         bias=vve[:, m:m + 1])
                    df = dfp.tile([128, P2], F32)
                    nc.vector.tensor_sub(df, a_, b_)
                    nc.vector.tensor_mul(df, df, wddr[:, m, :])
                    with nc.allow_low_precision(reason="20-elem reduce to f32r"):
                        nc.vector.reduce_sum(out=grs[:, m, 1:2], in_=df,
                                             axis=AX.X)
                    nc.scalar.copy(grs[:, m, 0:1], pq[:, P2:P2 + 1])
                pr = psr.tile([BC, 2], F32)
                for m in range(MR):
                    nc.tensor.matmul(pr, sel8, grs[:, m, :],
                                     start=(m == 0), stop=(m == MR - 1))
                nc.scalar.activation(gsb, pr[:, 0:1], AF.Sigmoid,
                                     bias=b3bc[:, 1:2], scale=1.0 / S)
                nc.scalar.activation(dsb, pr[:, 1:2], AF.Sigmoid,
                                     bias=b3bc[:, 2:3])

        # ---- LSTM over S steps + decoder ----
        if PHASES >= 5:
            NBORD = (0, 3, 1, 2)  # i, g, f, o: lets c-update overlap o-gates
            nc.vector.memset(hT.bitcast(DT.uint8), 0)
            with tc.tile_pool(name="psl", bufs=4, space="PSUM") as psl, \
                 tc.tile_pool(name="pstr", bufs=3, space="PSUM") as pstr, \
                 tc.tile_pool(name="psd", bufs=1, space="PSUM") as psd, \
                 tc.tile_pool(name="gap", bufs=8) as gap, \
                 tc.tile_pool(name="hp", bufs=2) as hp, \
                 tc.tile_pool(name="ltp", bufs=2) as ltp:
                for s in range(S):
                    m = s // 16
                    p0 = (s * 8) % 128
                    blk = (p0 // 64) * 64
                    q = (p0 % 64) // 8
                    # xg+bias fold first: independent of h, fills the PE gap
                    # left by the previous step's act/DVE tail.
                    pss = []
                    for nb in NBORD:
                        ps = psl.tile([32, 512], F32, tag="l")
                        nc.tensor.matmul(
                            ps, eyevar[blk:blk + 64, q, :],
                            xgsb[blk:blk + 64, m, nb * 512:(nb + 1) * 512],
                            start=True, stop=(s == 0))
                        pss.append(ps)
                    ga = {}
                    for ps, nb in zip(pss, NBORD):
                        if s > 0:
                            for j in range(KH // 2):
                                nc.tensor.matmul(
                                    ps, hT[:, 2 * j:2 * j + 2, :],
                                    whsb[:, 2 * j:2 * j + 2,
                                         nb * 512:(nb + 1) * 512],
                                    start=False, stop=(j == KH // 2 - 1),
                                    perf_mode=mybir.MatmulPerfMode.DoubleRow)
                        g = gap.tile([BC, 512], BF)
                        nc.scalar.activation(g, ps[0:BC, :],
                                             AF.Tanh if nb == 3 else AF.Sigmoid,
                                             scale=1.0 / WH_SCALE)
                        ga[nb] = g
                    if s == 0:
                        nc.vector.tensor_mul(cst, ga[0], ga[3])
                    else:
                        t1 = ltp.tile([BC, H], BF)
                        nc.vector.tensor_mul(t1, ga[0], ga[3])   # i*g
                        nc.vector.tensor_mul(cst, cst, ga[1])    # f*c
                        nc.vector.tensor_add(cst, cst, t1)
                    h = hp.tile([BC, H], BF)
                    nc.vector.tensor_mul(h, ga[2], cst)          # h = o*c
                    for k in range(KH):
                        pt = pstr.tile([128, BC], BF, tag="tr")
                        nc.tensor.transpose(pt, h[:, k * 128:(k + 1) * 128],
                                            eye8)
                        nc.vector.tensor_copy(hT[:, k, 0:BC], pt)
                # decoder
                pd = psd.tile([BC, 1], F32)
                for k in range(KH):
                    nc.tensor.matmul(pd, hT[:, k, 0:BC], wdecp[:, k:k + 1],
                                     start=(k == 0), stop=(k == KH - 1))
                nc.scalar.activation(osb, pd, AF.Sigmoid, bias=b3bc[:, 0:1],
                                     scale=1.0 / WH_SCALE)

        # ---- combine ----
        if PHASES >= 6:
            nc.vector.tensor_sub(fin, osb, dsb)
            nc.vector.scalar_tensor_tensor(fin, fin, gsb[:, 0:1], dsb,
                                           ALU.mult, ALU.add)
            nc.sync.dma_start(out_d, fin)


_NC_CACHE = {}


def _get_nc():
    if "nc" not in _NC_CACHE:
        _NC_CACHE["nc"] = build_nc()
    return _NC_CACHE["nc"]


def _make_in_maps(v_feat, category, W_enc, b_enc, Wx, bx, Wh, bh, cat_emb,
                  W_dec, b_dec, prototype, W_dd, b_dd, W_gate, b_gate):
    f32 = np.float32
    v_feat = np.asarray(v_feat, f32)
    category = np.asarray(category).astype(np.int64)

    wencp = np.ascontiguousarray(
        np.asarray(W_enc, f32).reshape(KF, 128, H).transpose(1, 0, 2)
    ).astype(BF_NP)
    benc = np.ascontiguousarray(
        np.asarray(b_enc, f32).reshape(KH, 128).T).copy()
    catemb = np.asarray(cat_emb, f32).astype(BF_NP)
    wxp = np.ascontiguousarray(
        np.asarray(Wx, f32).reshape(KH, 128, G).transpose(1, 0, 2)
    ).astype(BF_NP)
    bxh128 = np.ascontiguousarray(
        np.tile(WH_SCALE * (np.asarray(bx, f32)
                            + np.asarray(bh, f32)).reshape(1, G),
                (128, 1)))
    whp8 = np.ascontiguousarray(
        (WH_SCALE * np.asarray(Wh, f32)).reshape(KH, 128, G).transpose(1, 0, 2)
    ).astype(FP8_NP)
    # eyevar[p, q, j] = 1 iff p%64 == q*8+j (64-aligned step-row selector);
    # cols 8..31 are zero padding so the 32-wide PSUM region is fully started.
    pp_ = np.arange(128)
    eyevar = np.zeros((128, 8, 32), f32)
    for qq in range(8):
        for j in range(BC):
            eyevar[pp_ % 64 == qq * 8 + j, qq, j] = 1.0
    eyevar = eyevar.astype(BF_NP)
    eye8 = np.eye(BC, dtype=f32).astype(BF_NP)
    proto = np.asarray(prototype, f32)
    pw = np.concatenate([-2.0 * proto,
                         np.asarray(W_gate, f32).reshape(1, F),
                         np.zeros((1, F), f32)], axis=0)  # [22, F]
    pwT = np.ascontiguousarray(
        pw.T.reshape(KF, 128, 22).transpose(1, 0, 2)).astype(BF_NP)
    pprow = np.concatenate([(proto * proto).sum(axis=1),
                            np.zeros(2, f32)]).reshape(1, 22).astype(f32)
    ones1 = np.ones((1, 128), f32)
    wdd = np.asarray(W_dd, f32).reshape(S, P2)
    rep = np.repeat(wdd[:, None, :], BC, axis=1).reshape(R, P2)
    wddr = np.ascontiguousarray(rep.reshape(MR, 128, P2).transpose(1, 0, 2))
    sel8 = np.zeros((128, BC), f32)
    sel8[np.arange(128), np.arange(128) % BC] = 1.0
    wdecp8 = np.ascontiguousarray(
        (WH_SCALE * np.asarray(W_dec, f32)).reshape(KH, 128).T).astype(FP8_NP)
    b3 = np.array([np.asarray(b_dec, f32).reshape(-1)[0],
                   np.asarray(b_gate, f32).reshape(-1)[0],
                   np.asarray(b_dd, f32).reshape(-1)[0]], f32)
    b3bc = np.ascontiguousarray(np.tile(b3.reshape(1, 3), (BC, 1)))

    common = {
        "wencp": wencp, "benc": benc, "catemb": catemb, "wxp": wxp,
        "bxh128": bxh128, "whp8": whp8, "eyevar": eyevar, "eye8": eye8,
        "pwT": pwT, "pprow": pprow, "ones1": ones1, "wddr": wddr,
        "sel8": sel8, "wdecp8": wdecp8, "b3bc": b3bc,
    }
    in_maps = []
    for j in range(NCORES):
        vs = np.ascontiguousarray(
            v_feat[j * BC:(j + 1) * BC].transpose(1, 0, 2).reshape(R, F))
        vTn = np.ascontiguousarray(
            vs.reshape(R, KF, 128).transpose(2, 1, 0)).astype(BF_NP)
        vrow = np.ascontiguousarray(
            vs.reshape(MR, 128, F).transpose(1, 0, 2)).astype(BF_NP)
        cats = category[j * BC:(j + 1) * BC]
        onehot = (cats[None, :] == np.arange(3)[:, None]).astype(f32)
        onehot = np.ascontiguousarray(
            np.tile(onehot, (1, S))).astype(BF_NP)  # [3, R], r = s*8+b
        in_maps.append({"vT": vTn, "vrow": vrow, "onehot": onehot, **common})
    return in_maps


def run(trace=False, **inputs):
    nc = _get_nc()
    in_maps = _make_in_maps(**inputs)
    res = run_bass_kernel_spmd(nc, in_maps, list(range(NCORES)), trace=trace)
    out = np.concatenate([res.results[j]["out"] for j in range(NCORES)],
                         axis=0).astype(np.float32)
    return out, res


def kernel(**inputs):
    out, _ = run(trace=False, **inputs)
    return out
